# revision 1
# baseline (speedup 1.0000x reference)
# GATv2 two-layer GNN on 8 TRN2 cores — edge-major design.
#
# Per core: dst shard = contiguous 12500 nodes. Edges grouped by
# (src-window, dst-block) runs, each run padded to a cross-core common tile
# count (schedule identical on all cores -> single SPMD program).
# 128-edge tiles on partitions; per-edge math:
#   ax = axl~[src] + axr~[dst]      (|att|-scaled transforms, columns
#                                    sign-permuted: att>=0 first)
#   e0 = sum_{F+} lrelu(ax) - sum_{F-} lrelu(ax)  (per-dst const cancels in
#                                                  the segment softmax)
#   w  = exp(e0) * [xl | 1]
# Segment sum via one-hot fp8 matmuls (PE) accumulated in PSUM per run,
# evacuated (added) into an SBUF accumulator per dst-block; epilogue
# normalizes. Layer 2 identical, fed by a 2-chunk AllGather of [axl2~|xl2].
import numpy as np
import ml_dtypes
import concourse.bacc as bacc
import concourse.mybir as mybir
from concourse.tile import TileContext

F32 = mybir.dt.float32
BF16 = mybir.dt.bfloat16
FP8 = mybir.dt.float8e4
I16 = mybir.dt.int16
AL = mybir.AluOpType
AF = mybir.ActivationFunctionType
AX = mybir.AxisListType

BF = np.dtype(ml_dtypes.bfloat16)
F8 = mybir.dt.np(FP8)

N = 100000
NCORES = 8
PCORE = N // NCORES            # 12500
HALF = 6272
LOCAL = 2 * HALF               # 12544 padded local rows
NBLK = LOCAL // 128            # 98
CHALF = NCORES * HALF          # 50176 rows per collective half
TBL = 2 * CHALF                # 100352 global table rows
NWIN = 4
WROWS = TBL // NWIN            # 25088 (< 32767, int16-safe)
F_IN, HID, NCLS = 128, 64, 32
GT = 32                        # tiles per gather/compute group


def _wrap16(arr):
    a = np.asarray(arr, np.int16).reshape(-1, 16).T
    return np.tile(a, (8, 1))


def _r2(s):
    """Global node id -> padded table row (collective-half-major layout)."""
    c = s // PCORE
    i = s % PCORE
    h = i // HALF
    return h * CHALF + c * HALF + (i - h * HALF)


class Plan:
    pass


def build_plan(edge_index, n_nodes):
    assert n_nodes == N
    src = np.asarray(edge_index[0], np.int64)
    dst = np.asarray(edge_index[1], np.int64)
    loops = np.arange(n_nodes, dtype=np.int64)
    src = np.concatenate([src, loops])
    dst = np.concatenate([dst, loops])

    p = Plan()
    # bucket edges per (core, window, block)
    buckets = [[[None] * NBLK for _ in range(NWIN)] for _ in range(NCORES)]
    cnt = np.zeros((NCORES, NWIN, NBLK), np.int64)
    core = dst // PCORE
    dl_all = dst - core * PCORE
    r2_all = _r2(src)
    w_all = r2_all // WROWS
    b_all = dl_all // 128
    for c in range(NCORES):
        m = core == c
        dlc = dl_all[m]
        r2c = r2_all[m]
        wc = w_all[m]
        bc = b_all[m]
        for wi in range(NWIN):
            mw = wc == wi
            dlw = dlc[mw]
            r2w = r2c[mw]
            bw = bc[mw]
            o = np.argsort(bw, kind="stable")
            dlw, r2w, bw = dlw[o], r2w[o], bw[o]
            # split by block
            bounds = np.searchsorted(bw, np.arange(NBLK + 1))
            for b in range(NBLK):
                s0, s1 = bounds[b], bounds[b + 1]
                buckets[c][wi][b] = (dlw[s0:s1], r2w[s0:s1])
                cnt[c, wi, b] = s1 - s0

    # common run lengths (tiles) per (window, block)
    R = np.ceil(cnt.max(axis=0) / 128).astype(np.int64)   # [NWIN, NBLK]
    R = np.maximum(R, 1)
    p.R = R
    p.NTW = [int(R[wi].sum()) for wi in range(NWIN)]
    NT = sum(p.NTW)
    p.NT = NT

    # block-major tile schedule: for b: for w: R[w,b] tiles.
    # tile metadata common to all cores.
    p.tile_w = []
    p.tile_b = []
    p.tile_st = []
    p.tile_sp = []
    for b in range(NBLK):
        first = True
        for wi in range(NWIN):
            for r in range(int(R[wi, b])):
                p.tile_w.append(wi)
                p.tile_b.append(b)
                p.tile_st.append(first)
                first = False
                p.tile_sp.append(False)
        p.tile_sp[-1] = True
    assert len(p.tile_w) == NT

    # per-core padded lane arrays + M stream, in schedule order
    p.idxA, p.idxR, p.M = [], [], []
    for c in range(NCORES):
        A = np.zeros((NT, 128), np.int16)
        Rr = np.zeros((NT, 128), np.int16)
        M = np.zeros((128, NT * 128), F8)
        t = 0
        for b in range(NBLK):
            for wi in range(NWIN):
                dlw, r2w = buckets[c][wi][b]
                n = len(dlw)
                for r in range(int(R[wi, b])):
                    lo, hi = r * 128, min((r + 1) * 128, n)
                    k = max(0, hi - lo)
                    if k > 0:
                        A[t, :k] = (r2w[lo:lo + k] - wi * WROWS).astype(np.int16)
                        Rr[t, :k] = dlw[lo:lo + k].astype(np.int16)
                        Mp = np.zeros((128, 128), np.float32)
                        Mp[np.arange(k), dlw[lo:lo + k] - 128 * b] = 1.0
                        M[:, t * 128:(t + 1) * 128] = Mp.astype(F8)
                    t += 1
        assert t == NT
        p.idxA.append(_wrap16(A.ravel()))
        p.idxR.append(_wrap16(Rr.ravel()))
        p.M.append(M)
    return p


def build_inputs(p, x, W1l, b1l, W1r, b1r, att1, bias1,
                 W2l, b2l, W2r, b2r, att2, bias2):
    x = np.asarray(x, np.float32)
    att1 = np.asarray(att1, np.float32)
    att2 = np.asarray(att2, np.float32)

    p1 = np.argsort(att1 < 0, kind="stable")
    p2 = np.argsort(att2 < 0, kind="stable")
    p.NPOS1 = int((att1 >= 0).sum())
    p.NPOS2 = int((att2 >= 0).sum())
    p.ZBIAS1 = bool(np.all(np.asarray(b1l, np.float32) == 0))
    p.p2 = p2

    W1la = (np.asarray(W1l, np.float32) * np.abs(att1))[:, p1]
    b1la = (np.asarray(b1l, np.float32) * np.abs(att1))[p1]
    W1ra = (np.asarray(W1r, np.float32) * np.abs(att1))[:, p1]
    b1ra = (np.asarray(b1r, np.float32) * np.abs(att1))[p1]
    W2la = (np.asarray(W2l, np.float32) * np.abs(att2))[:, p2]
    W2ra = (np.asarray(W2r, np.float32) * np.abs(att2))[:, p2]
    b2ra = (np.asarray(b2r, np.float32) * np.abs(att2))[p2]

    r2 = _r2(np.arange(N))
    xT = np.zeros((F_IN, TBL), BF)
    xT[:, r2] = x.T.astype(BF)

    def brow(v):
        return np.broadcast_to(np.asarray(v, np.float32), (128, len(v))).copy()

    base = {
        "xT": xT,
        "W1la": W1la.astype(BF), "W1l": np.asarray(W1l, np.float32).astype(BF),
        "W1ra": W1ra.astype(BF),
        "b1comb_r": np.concatenate([brow(b1la), brow(np.asarray(b1l, np.float32))], axis=1),
        "b1ra_r": brow(b1ra),
        "bias1_r": brow(np.asarray(bias1, np.float32)),
        "W2l": np.asarray(W2l, np.float32)[:, p2].astype(BF),
        "W2la_c": W2la.astype(BF),
        "W2ra": W2ra.astype(BF),
        "b2l_r": brow(np.asarray(b2l, np.float32)[p2]),
        "b2comb_r": np.concatenate([brow(np.asarray(b2l, np.float32)[p2]), brow(b2ra)], axis=1),
        "bias2_r": brow(np.asarray(bias2, np.float32)[p2]),
        "att2a_r": brow(np.abs(att2)[p2]),
        "ident": np.eye(128, dtype=np.float32),
    }

    ins = []
    for c in range(NCORES):
        m = dict(base)
        xs = np.zeros((F_IN, LOCAL), BF)
        xs[:, :PCORE] = x[c * PCORE:(c + 1) * PCORE].T.astype(BF)
        m["xTs"] = xs
        m["idxA"] = p.idxA[c]
        m["idxR"] = p.idxR[c]
        m["M"] = p.M[c]
        ins.append(m)
    return ins


def build_nc(p, stop_after=None):
    NT = p.NT
    NP1, NP2 = p.NPOS1, p.NPOS2
    R = p.R

    nc = bacc.Bacc()
    dp = nc.declare_dram_parameter
    xT_d = dp("xT", [F_IN, TBL], BF16, isOutput=False)
    xTs_d = dp("xTs", [F_IN, LOCAL], BF16, isOutput=False)
    W1la_d = dp("W1la", [F_IN, HID], BF16, isOutput=False)
    W1l_d = dp("W1l", [F_IN, HID], BF16, isOutput=False)
    W1ra_d = dp("W1ra", [F_IN, HID], BF16, isOutput=False)
    b1comb_d = dp("b1comb_r", [128, 2 * HID], F32, isOutput=False)
    b1ra_d = dp("b1ra_r", [128, HID], F32, isOutput=False)
    bias1_d = dp("bias1_r", [128, HID], F32, isOutput=False)
    W2l_d = dp("W2l", [HID, NCLS], BF16, isOutput=False)
    W2la_d = dp("W2la_c", [HID, NCLS], BF16, isOutput=False)
    W2ra_d = dp("W2ra", [HID, NCLS], BF16, isOutput=False)
    b2l_d = dp("b2l_r", [128, NCLS], F32, isOutput=False)
    b2comb_d = dp("b2comb_r", [128, 2 * NCLS], F32, isOutput=False)
    bias2_d = dp("bias2_r", [128, NCLS], F32, isOutput=False)
    att2a_d = dp("att2a_r", [128, NCLS], F32, isOutput=False)
    ident_d = dp("ident", [128, 128], F32, isOutput=False)
    idxA_d = dp("idxA", [128, NT * 8], I16, isOutput=False)
    idxR_d = dp("idxR", [128, NT * 8], I16, isOutput=False)
    M_d = dp("M", [128, NT * 128], FP8, isOutput=False)
    out_d = dp("out2", [LOCAL, NCLS], F32, isOutput=True)

    tab1 = nc.dram_tensor("tab1", [TBL, 128], BF16)          # [axl1~|xl1]
    axr1 = nc.dram_tensor("axr1", [LOCAL, HID], F32)
    hl2_loc = nc.dram_tensor("hl2_loc", [LOCAL, NCLS], BF16)     # xl2 only
    hlA = nc.dram_tensor("hlA", [CHALF, NCLS], BF16, addr_space="Shared")
    hlB = nc.dram_tensor("hlB", [CHALF, NCLS], BF16, addr_space="Shared")
    tab2 = nc.dram_tensor("tab2", [TBL, 2 * NCLS], F32)      # [axl2~|xl2]
    axr2 = nc.dram_tensor("axr2", [LOCAL, 2 * NCLS], F32)    # [axr2~|0]

    with TileContext(nc) as tc:
        with (
            tc.tile_pool(name="const", bufs=1) as cpool,
            tc.tile_pool(name="xa", bufs=3) as xapool,
            tc.tile_pool(name="pa", bufs=2, space="PSUM") as papool,
            tc.tile_pool(name="chain", bufs=3, space="PSUM") as chpool,
            tc.tile_pool(name="pe2", bufs=1, space="PSUM") as pe2pool,
            tc.tile_pool(name="st", bufs=2) as stpool,
            tc.tile_pool(name="ms", bufs=2) as mspool,
            tc.tile_pool(name="gx", bufs=2) as gxpool,
            tc.tile_pool(name="wk", bufs=2) as wkpool,
            tc.tile_pool(name="ep", bufs=3) as eppool,
        ):
            dqs = [nc.sync, nc.scalar]
            _dq = [0]
            def dq():
                _dq[0] += 1
                return dqs[_dq[0] % 2]

            def cload(dram, shape, dt):
                nm = dram.name + "_s"
                t = cpool.tile(shape, dt, name=nm, tag=nm)
                nc.sync.dma_start(out=t[:], in_=dram[:])
                return t
            W1la_s = cload(W1la_d, [F_IN, HID], BF16)
            W1l_s = cload(W1l_d, [F_IN, HID], BF16)
            W1ra_s = cload(W1ra_d, [F_IN, HID], BF16)
            b1comb_s = cload(b1comb_d, [128, 2 * HID], F32)
            b1ra_s = cload(b1ra_d, [128, HID], F32)
            bias1_s = cload(bias1_d, [128, HID], F32)
            W2l_s = cload(W2l_d, [HID, NCLS], BF16)
            W2la_s = cload(W2la_d, [HID, NCLS], BF16)
            W2ra_s = cload(W2ra_d, [HID, NCLS], BF16)
            b2l_s = cload(b2l_d, [128, NCLS], F32)
            b2comb_s = cload(b2comb_d, [128, 2 * NCLS], F32)
            bias2_s = cload(bias2_d, [128, NCLS], F32)
            att2a_s = cload(att2a_d, [128, NCLS], F32)
            ident = cload(ident_d, [128, 128], F32)

            zeros32 = cpool.tile([128, NCLS], F32)
            nc.vector.memset(zeros32[:], 0.0)

            # ---- phase A ----
            TB = 4
            for j in range(0, (TBL // 128 if stop_after != "consts" else 0), TB):
                nt = min(TB, TBL // 128 - j)
                xt = xapool.tile([128, TB * 128], BF16, tag="xt")
                dq().dma_start(out=xt[:, :nt * 128], in_=xT_d[:, j * 128:(j + nt) * 128])
                ot = xapool.tile([128, TB * 128], BF16, tag="ot")
                for b in range(nt):
                    psA = papool.tile([128, 128], F32, tag="psA")
                    nc.tensor.matmul(out=psA[:, 0:HID], lhsT=xt[:, b * 128:(b + 1) * 128],
                                     rhs=W1la_s[:], start=True, stop=True,
                                     skip_group_check=True)
                    nc.tensor.matmul(out=psA[:, HID:128], lhsT=xt[:, b * 128:(b + 1) * 128],
                                     rhs=W1l_s[:], start=True, stop=True,
                                     skip_group_check=True)
                    if p.ZBIAS1 and (j + b) % 2 == 0:
                        nc.scalar.activation(out=ot[:, b * 128:(b + 1) * 128],
                                             in_=psA[:], func=AF.Copy)
                    else:
                        nc.vector.tensor_tensor(out=ot[:, b * 128:(b + 1) * 128],
                                                in0=psA[:], in1=b1comb_s[:], op=AL.add)
                dq().dma_start(
                    out=tab1[j * 128:(j + nt) * 128, :].rearrange("(b q) f -> q b f", q=128),
                    in_=ot[:, :nt * 128].rearrange("q (b f) -> q b f", f=128))
            for j in range(0, (LOCAL // 128 if stop_after != "consts" else 0), TB):
                nt = min(TB, LOCAL // 128 - j)
                xt = xapool.tile([128, TB * 128], BF16, tag="xt")
                dq().dma_start(out=xt[:, :nt * 128], in_=xTs_d[:, j * 128:(j + nt) * 128])
                ot = xapool.tile([128, TB * HID], F32, tag="otr")
                for b in range(nt):
                    psA = papool.tile([128, 128], F32, tag="psA")
                    nc.tensor.matmul(out=psA[:, 0:HID], lhsT=xt[:, b * 128:(b + 1) * 128],
                                     rhs=W1ra_s[:], start=True, stop=True,
                                     skip_group_check=True)
                    nc.vector.tensor_tensor(out=ot[:, b * HID:(b + 1) * HID],
                                            in0=psA[:, 0:HID], in1=b1ra_s[:], op=AL.add)
                dq().dma_start(
                    out=axr1[j * 128:(j + nt) * 128, :].rearrange("(b q) f -> q b f", q=128),
                    in_=ot[:, :nt * HID].rearrange("q (b f) -> q b f", f=HID))

            # ---- edge sweep (block-major, inline epilogue) ----
            def edge_sweep(tab, tabw, tabdt, F, NPOS, axr_t, axrw, layer, epi, nomm=False, gonly=False):
                ICH = 128
                cur = {"i": None, "ia": None, "ir": None}
                chain_ps = [None]
                NG = (NT + GT - 1) // GT
                for g in range(NG):
                    tile0 = g * GT
                    gn = min(GT, NT - tile0)
                    ich = tile0 // ICH
                    if cur["i"] != ich:
                        iat = stpool.tile([128, ICH * 8], I16, tag=f"ia{layer}",
                                          name="iat")
                        irt = stpool.tile([128, ICH * 8], I16, tag=f"ir{layer}",
                                          name="irt")
                        n = min(ICH, NT - ich * ICH)
                        nc.scalar.dma_start(
                            out=iat[:, :n * 8],
                            in_=idxA_d[:, ich * ICH * 8:(ich * ICH + n) * 8])
                        nc.scalar.dma_start(
                            out=irt[:, :n * 8],
                            in_=idxR_d[:, ich * ICH * 8:(ich * ICH + n) * 8])
                        cur["i"], cur["ia"], cur["ir"] = ich, iat, irt
                    io0 = (tile0 % ICH) * 8
                    import os as _os
                    mt = mspool.tile([128, GT * 128], FP8, tag=f"M{layer}", name="mt")
                    if _os.environ.get("SKIPM") != "1":
                        nc.scalar.dma_start(out=mt[:, :gn * 128],
                                        in_=M_d[:, tile0 * 128:(tile0 + gn) * 128])
                    gx = gxpool.tile([128, GT * tabw], tabdt, tag=f"gx{layer}")
                    s0 = gn if _os.environ.get("SKIPG") == "1" else 0
                    while s0 < gn:
                        wseg = p.tile_w[tile0 + s0]
                        s1 = s0
                        while s1 < gn and p.tile_w[tile0 + s1] == wseg and s1 - s0 < 8:
                            s1 += 1
                        nc.gpsimd.dma_gather(
                            out_ap=gx[:, s0 * tabw:s1 * tabw]
                                .rearrange("q (c f) -> q c f", f=tabw),
                            in_ap=tab[wseg * WROWS:(wseg + 1) * WROWS, :],
                            idxs_ap=cur["ia"][:, io0 + s0 * 8:io0 + s1 * 8],
                            num_idxs=(s1 - s0) * 128, num_idxs_reg=(s1 - s0) * 128,
                            elem_size=tabw)
                        s0 = s1
                    gr = gxpool.tile([128, GT * axrw], F32, tag=f"gr{layer}")
                    if _os.environ.get("SKIPG") != "1":
                        for r0 in range(0, gn, 8):
                            r1 = min(r0 + 8, gn)
                            nc.gpsimd.dma_gather(
                                out_ap=gr[:, r0 * axrw:r1 * axrw]
                                    .rearrange("q (c f) -> q c f", f=axrw),
                                in_ap=axr_t[:, :],
                                idxs_ap=cur["ir"][:, io0 + r0 * 8:io0 + r1 * 8],
                                num_idxs=(r1 - r0) * 128, num_idxs_reg=(r1 - r0) * 128,
                                elem_size=axrw)
                    if gonly:
                        continue
                    gx3 = gx[:, :gn * tabw].rearrange("q (c f) -> q c f", f=tabw)
                    gr3 = gr[:, :gn * axrw].rearrange("q (c f) -> q c f", f=axrw)
                    ax = wkpool.tile([128, GT * F], BF16, tag=f"ax{layer}")
                    ax3 = ax[:, :gn * F].rearrange("q (c f) -> q c f", f=F)
                    adde = nc.vector if g % 2 else nc.gpsimd
                    adde.tensor_tensor(out=ax3, in0=gx3[:, :, 0:F],
                                       in1=gr3[:, :, 0:F], op=AL.add)
                    lr = wkpool.tile([128, GT * F], BF16, tag=f"lr{layer}")
                    nc.vector.scalar_tensor_tensor(
                        out=lr[:, :gn * F], in0=ax[:, :gn * F], scalar=0.2,
                        in1=ax[:, :gn * F], op0=AL.mult, op1=AL.max)
                    lr3 = lr[:, :gn * F].rearrange("q (c f) -> q c f", f=F)
                    rp = wkpool.tile([128, GT], F32, tag=f"rp{layer}")
                    nc.vector.tensor_reduce(out=rp[:, :gn].unsqueeze(2),
                                            in_=lr3[:, :, 0:NPOS], axis=AX.X, op=AL.add)
                    e0 = wkpool.tile([128, GT], F32, tag=f"e0{layer}")
                    if NPOS < F:
                        rm = wkpool.tile([128, GT], F32, tag=f"rm{layer}")
                        nc.vector.tensor_reduce(out=rm[:, :gn].unsqueeze(2),
                                                in_=lr3[:, :, NPOS:F], axis=AX.X,
                                                op=AL.add)
                        nc.vector.tensor_tensor(out=e0[:, :gn], in0=rp[:, :gn],
                                                in1=rm[:, :gn], op=AL.subtract)
                    else:
                        nc.vector.tensor_copy(out=e0[:, :gn], in_=rp[:, :gn])
                    pe = wkpool.tile([128, GT], BF16, tag=f"pe{layer}")
                    nc.scalar.activation(out=pe[:, :gn], in_=e0[:, :gn], func=AF.Exp)
                    wt = wkpool.tile([128, GT * (F + 1)], BF16, tag=f"wt{layer}")
                    wt3 = wt[:, :gn * (F + 1)].rearrange("q (c f) -> q c f", f=F + 1)
                    if layer == 1:
                        wex = wkpool.tile([128, GT * F], BF16, tag=f"wex{layer}")
                        nc.scalar.activation(
                            out=wex[:, :gn * F].rearrange("q (c f) -> q c f", f=F),
                            in_=pe[:, :gn].unsqueeze(2).to_broadcast([128, gn, F]),
                            func=AF.Copy)
                        nc.vector.tensor_tensor(
                            out=wt3[:, :, 0:F], in0=gx3[:, :, F:2 * F],
                            in1=wex[:, :gn * F].rearrange("q (c f) -> q c f", f=F),
                            op=AL.mult)
                    else:
                        mule = nc.gpsimd if g % 2 else nc.vector
                        mule.tensor_tensor(
                            out=wt3[:, :, 0:F], in0=gx3[:, :, F:2 * F],
                            in1=pe[:, :gn].unsqueeze(2).to_broadcast([128, gn, F]),
                            op=AL.mult)
                    nc.vector.tensor_copy(out=wt3[:, :, F:F + 1],
                                          in_=pe[:, :gn].unsqueeze(2))
                    for rel in range(gn if not nomm else 0):
                        ti = tile0 + rel
                        if p.tile_st[ti]:
                            chain_ps[0] = chpool.tile([128, F + 1], F32,
                                                      tag="ch", name=f"ch{layer}")
                        ps = chain_ps[0]
                        nc.tensor.matmul(
                            out=ps[:], lhsT=mt[:, rel * 128:(rel + 1) * 128],
                            rhs=wt[:, rel * (F + 1):(rel + 1) * (F + 1)],
                            start=p.tile_st[ti], stop=p.tile_sp[ti])
                        if p.tile_sp[ti]:
                            epi(p.tile_b[ti], ps)

            # ---- epilogue callbacks ----
            def ep1(b, a):
                den = eppool.tile([128, 1], F32, tag="den", name="den")
                nc.vector.tensor_scalar(out=den[:], in0=a[:, HID:HID + 1],
                                        scalar1=1e-16, scalar2=None, op0=AL.add)
                rec = eppool.tile([128, 1], F32, tag="rec", name="rec")
                nc.vector.reciprocal(out=rec[:], in_=den[:])
                hb = eppool.tile([128, HID], F32, tag="hb", name="hb")
                nc.vector.tensor_tensor(out=hb[:], in0=a[:, 0:HID],
                                        in1=rec[:].to_broadcast([128, HID]), op=AL.mult)
                nc.gpsimd.tensor_tensor(out=hb[:], in0=hb[:], in1=bias1_s[:], op=AL.add)
                h = eppool.tile([128, HID], F32, tag="h", name="h")
                nc.scalar.activation(out=h[:], in_=hb[:], func=AF.Relu)
                pst = pe2pool.tile([HID, 128], F32, tag="psT", name="pst")
                nc.tensor.transpose(out=pst[:], in_=h[:], identity=ident[:])
                hT = eppool.tile([HID, 128], BF16, tag="hT", name="hT")
                nc.vector.tensor_copy(out=hT[:], in_=pst[:])
                ps2 = pe2pool.tile([128, 2 * NCLS], F32, tag="ps2", name="ps2")
                nc.tensor.matmul(out=ps2[:, 0:NCLS], lhsT=hT[:], rhs=W2l_s[:],
                                 start=True, stop=True, skip_group_check=True)
                nc.tensor.matmul(out=ps2[:, NCLS:2 * NCLS], lhsT=hT[:], rhs=W2ra_s[:],
                                 start=True, stop=True, skip_group_check=True)
                both = eppool.tile([128, 2 * NCLS], F32, tag="both", name="both")
                nc.vector.tensor_tensor(out=both[:], in0=ps2[:], in1=b2comb_s[:],
                                        op=AL.add)
                hl2 = eppool.tile([128, NCLS], BF16, tag="hl2", name="hl2")
                nc.vector.tensor_copy(out=hl2[:], in_=both[:, 0:NCLS])
                dq().dma_start(out=hl2_loc[b * 128:(b + 1) * 128, :], in_=hl2[:])
                ar2 = eppool.tile([128, 2 * NCLS], F32, tag="ar2", name="ar2")
                nc.vector.tensor_copy(out=ar2[:, 0:NCLS], in_=both[:, NCLS:2 * NCLS])
                nc.vector.tensor_copy(out=ar2[:, NCLS:2 * NCLS], in_=zeros32[:])
                dq().dma_start(out=axr2[b * 128:(b + 1) * 128, :], in_=ar2[:])
                if b == NBLK // 2 - 1 and (stop_after is None or stop_after in ("col", "exp")):
                    nc.gpsimd.collective_compute(
                        "AllGather", AL.bypass,
                        replica_groups=[list(range(NCORES))],
                        ins=[hl2_loc[0:HALF, :]], outs=[hlA[:, :]])

            def ep2(b, a):
                den = eppool.tile([128, 1], F32, tag="den", name="den")
                nc.vector.tensor_scalar(out=den[:], in0=a[:, NCLS:NCLS + 1],
                                        scalar1=1e-16, scalar2=None, op0=AL.add)
                rec = eppool.tile([128, 1], F32, tag="rec", name="rec")
                nc.vector.reciprocal(out=rec[:], in_=den[:])
                ob = eppool.tile([128, NCLS], F32, tag="ob", name="ob")
                nc.vector.tensor_tensor(out=ob[:], in0=a[:, 0:NCLS],
                                        in1=rec[:].to_broadcast([128, NCLS]), op=AL.mult)
                nc.gpsimd.tensor_tensor(out=ob[:], in0=ob[:], in1=bias2_s[:], op=AL.add)
                dq().dma_start(out=out_d[b * 128:(b + 1) * 128, :], in_=ob[:])

            if stop_after is not None:
                zf = eppool.tile([128, NCLS], F32, tag="ob", name="zf")
                nc.vector.tensor_copy(out=zf[:], in_=zeros32[:])
                nc.sync.dma_start(out=out_d[0:128, :], in_=zf[:])
            if stop_after == "l1nomm":
                edge_sweep(tab1, 128, BF16, HID, NP1, axr1, HID, 1, ep1, nomm=True)
            if stop_after == "gonly":
                edge_sweep(tab1, 128, BF16, HID, NP1, axr1, HID, 1, ep1, nomm=True, gonly=True)
            if stop_after is None or stop_after in ("l1", "ep1", "col", "exp"):
                edge_sweep(tab1, 128, BF16, HID, NP1, axr1, HID, 1, ep1)
            if stop_after is None or stop_after in ("col", "exp"):
                nc.gpsimd.collective_compute(
                    "AllGather", AL.bypass,
                    replica_groups=[list(range(NCORES))],
                    ins=[hl2_loc[HALF:LOCAL, :]], outs=[hlB[:, :]])

            # ---- expand halves into f32 tab2 ----
            TB2 = 8
            for half, hl in (((0, hlA), (1, hlB)) if (stop_after is None or stop_after == "exp") else ()):
                for j in range(0, CHALF // 128, TB2):
                    nt = min(TB2, CHALF // 128 - j)
                    gt_ = xapool.tile([128, TB2 * NCLS], BF16, tag="g2")
                    dq().dma_start(
                        out=gt_[:, :nt * NCLS].rearrange("q (b f) -> q b f", f=NCLS),
                        in_=hl[j * 128:(j + nt) * 128, :].rearrange("(b q) f -> q b f", q=128))
                    ot = xapool.tile([128, TB2 * 2 * NCLS], F32, tag="o2")
                    ot3 = ot[:, :nt * 2 * NCLS].rearrange("q (b f) -> q b f", f=2 * NCLS)
                    gt3 = gt_[:, :nt * NCLS].rearrange("q (b f) -> q b f", f=NCLS)
                    a2b = att2a_s[:].unsqueeze(1).to_broadcast([128, nt, NCLS])
                    nc.vector.tensor_tensor(out=ot3[:, :, 0:NCLS], in0=gt3, in1=a2b,
                                            op=AL.mult)
                    nc.scalar.activation(out=ot3[:, :, NCLS:2 * NCLS], in_=gt3,
                                         func=AF.Copy)
                    dq().dma_start(
                        out=tab2[half * CHALF + j * 128: half * CHALF + (j + nt) * 128, :]
                            .rearrange("(b q) f -> q b f", q=128),
                        in_=ot[:, :nt * 2 * NCLS].rearrange("q (b f) -> q b f", f=2 * NCLS))

            if stop_after is None:
                edge_sweep(tab2, 2 * NCLS, F32, NCLS, NP2, axr2, 2 * NCLS, 2, ep2)

    nc.compile()
    return nc


def unshard_output(p, results):
    out = np.zeros((N, NCLS), np.float32)
    for c in range(NCORES):
        out[c * PCORE:(c + 1) * PCORE, p.p2] = results[c]["out2"][:PCORE]
    return out


_NC_CACHE = {}


def _get_nc(p):
    key = (tuple(p.NTW), p.NPOS1, p.NPOS2, p.ZBIAS1, tuple(p.R.ravel().tolist()))
    if key not in _NC_CACHE:
        _NC_CACHE.clear()
        _NC_CACHE[key] = build_nc(p)
    return _NC_CACHE[key]


def kernel(x, edge_index, W1l, b1l, W1r, b1r, att1, bias1,
           W2l, b2l, W2r, b2r, att2, bias2):
    from concourse.bass_utils import run_bass_kernel_spmd
    x = np.asarray(x)
    edge_index = np.asarray(edge_index)
    p = build_plan(edge_index, x.shape[0])
    ins = build_inputs(p, x, W1l, b1l, W1r, b1r, att1, bias1,
                       W2l, b2l, W2r, b2r, att2, bias2)
    nc = _get_nc(p)
    res = run_bass_kernel_spmd(nc, ins, core_ids=list(range(NCORES)))
    return unshard_output(p, [res.results[c] for c in range(NCORES)])

